# revision 16
# baseline (speedup 1.0000x reference)
"""Trainium2 Bass kernel for nn_ActionDeltaMPredictor.

Pipeline per token: LayerNorm -> SwiGLU (H=1024 hidden) -> Linear to 6x6 ->
skew-symmetrize -> Frobenius clip -> matrix exponential (orthogonal output).

Sharding: pure data parallel over 8 NeuronCores; weights replicated.

Kernel structure per core (B_shard = 16384 tokens = 128 tiles of 128).
The PE is the bottleneck (fp16 matmul floor ~930us); everything else is
scheduled to keep it saturated:
  - Weights are DMAed in 128-row chunks on the gpsimd queue so the first
    matmul only waits for one 256KB chunk, not 4MB.
  - Feeder per 128-token tile: DMA x (fp16) on sync, bn_stats LayerNorm
    stats + rsqrt bit-trick on DVE, then the normalize runs on the
    SCALAR engine (Identity activation with per-partition scale/bias:
    (x-mu)*rstd), and the XBAR transpose to feature-major issues from
    the SYNC queue.  DVE only carries stats + h-mults.
  - SwiGLU per 512-token group at N=512; Silu on scalar.  The head
    matmul for chunk j0-1 is interleaved right after val chunk j0 so
    the h-dependency is spread through the group instead of bunching
    at the group end.
  - Head: A_skew = h @ ws with the skew fold in the host-prepped
    weights; PSUM->fp16 copy on scalar, XBAR transpose into the
    token-major W tile from sync.
  - Expm per batch (Frobenius clip + degree-6 Taylor + 2 squarings in
    fp16): the per-token 6x6 ops are split into two INDEPENDENT
    token-range chains, one on DVE and one on gpsimd (no cross-engine
    handoffs), with tree-structured adds to shorten the serial chain.
    Ops are paced into the next batch's groups after each j0 step so
    they never sit ahead of h-mults in the DVE queue.
  - Batch sizes taper (7x16, 8, 4, 4 tiles) so the expm of the final
    batch (which cannot overlap any matmul) is small.
"""

import numpy as np
import os

_F = lambda name, dflt='1': os.environ.get(name, dflt) == '1'
KOPT_NORM_ACT = _F('KOPT_NORM_ACT')
KOPT_TP_SYNC = _F('KOPT_TP_SYNC', '0')
KOPT_W_GP = _F('KOPT_W_GP')
KOPT_A16_ACT = _F('KOPT_A16_ACT')
KOPT_HEAD_IL = _F('KOPT_HEAD_IL')

P = 128
H = 1024
KC = H // P          # contraction chunks
JC = H // P          # hidden chunks
M = 36               # 6x6
MP = 48              # padded slot stride (XBAR tile rows = 16)
GROUP = 4            # token tiles per matmul group
TG = P * GROUP       # tokens per group
LN_EPS = 1e-5
MAX_NORM = 3.0
N_SQ = 2             # squarings (degree-6 Taylor)
INV_SCALE = 1.0 / (1 << N_SQ)
MAGIC = 0x5F3759DF   # rsqrt seed
EG_MAX = 16

N_CORES = 8
B_FULL = 131072
B_SHARD = B_FULL // N_CORES


def _fap(t, elem_off, dims):
    """AP over tile t with custom free dims [[step, count], ...]."""
    import concourse.bass as bass
    return bass.AP(
        tensor=t.tensor,
        offset=t.offset + elem_off,
        ap=[list(t.ap[0])] + [list(d) for d in dims],
    )


def build(nc, b_shard, egs=None, has_bias=False, silu_mode='act'):
    """Emit the kernel IR into Bass object nc."""
    import concourse.tile as tile
    import concourse.mybir as mybir

    f32 = mybir.dt.float32
    f16 = mybir.dt.float16
    i32 = mybir.dt.int32
    Alu = mybir.AluOpType
    Act = mybir.ActivationFunctionType

    NT = b_shard // P          # token tiles
    if egs is None:
        egs = [16] * ((NT - 16) // 16) + [8, 4, 4]
    assert sum(egs) == NT and all(eg % GROUP == 0 for eg in egs)
    NGT = NT // GROUP          # total groups
    # v-engine slot share per batch size (DVE ~2.9x faster per elem than
    # gpsimd, but DVE also carries feeder+h work mid-kernel)
    SV = {16: 12, 8: 6, 4: 3}

    x_d = nc.dram_tensor("x", [b_shard, H], f16, kind="ExternalInput")
    wg_d = nc.dram_tensor("wgT", [H, H], f16, kind="ExternalInput")
    wv_d = nc.dram_tensor("wvT", [H, H], f16, kind="ExternalInput")
    ws_d = nc.dram_tensor("wsT", [H, M], f16, kind="ExternalInput")
    if has_bias:
        bg_d = nc.dram_tensor("bg", [H], f32, kind="ExternalInput")
        bv_d = nc.dram_tensor("bv", [H], f32, kind="ExternalInput")
    out_d = nc.dram_tensor("out", [b_shard, M], f32, kind="ExternalOutput")

    from contextlib import ExitStack
    with tile.TileContext(nc) as tc, ExitStack() as ctx:
        singles = ctx.enter_context(tc.tile_pool(name="singles", bufs=1))
        xp = ctx.enter_context(tc.tile_pool(name="xp", bufs=2))
        statp = ctx.enter_context(tc.tile_pool(name="statp", bufs=2))
        xhp = ctx.enter_context(tc.tile_pool(name="xhp", bufs=2))
        xtap = ctx.enter_context(tc.tile_pool(name="xtap", bufs=2))
        hp = ctx.enter_context(tc.tile_pool(name="hp", bufs=2))
        sgp = ctx.enter_context(tc.tile_pool(name="sgp", bufs=2))
        wp = ctx.enter_context(tc.tile_pool(name="wp", bufs=2))
        ep = ctx.enter_context(tc.tile_pool(name="ep", bufs=2))
        efp = ctx.enter_context(tc.tile_pool(name="efp", bufs=2))
        ps_mm = ctx.enter_context(tc.tile_pool(name="ps_mm", bufs=6, space="PSUM"))
        ps_pa = ctx.enter_context(tc.tile_pool(name="ps_pa", bufs=2, space="PSUM"))

        v = nc.vector
        s = nc.scalar
        gp = nc.gpsimd

        # ---- persistent weights, chunked 2 k-blocks per DMA: few DMA
        # instructions (semaphore-rotation pressure) but still fine-
        # grained enough that the first matmuls start early ----
        wg_ch = [singles.tile([P, 2, H], f16, name=f"wgc{c}") for c in range(4)]
        wv_ch = [singles.tile([P, 2, H], f16, name=f"wvc{c}") for c in range(4)]
        wg_sb = [wg_ch[k // 2][:, k % 2, :] for k in range(KC)]
        wv_sb = [wv_ch[k // 2][:, k % 2, :] for k in range(KC)]
        ws_t = singles.tile([P, JC, M], f16, name="ws_t")
        ws_sb = [ws_t[:, j, :] for j in range(JC)]

        def load_weights_main():
            # gpsimd queue: transfers run in parallel with the x loads on
            # sync (each queue only sustains ~95GB/s)
            for c in range(4):
                gp.dma_start(
                    wg_ch[c][:, :, :],
                    wg_d[c * 2 * P:(c + 1) * 2 * P, :].rearrange(
                        "(c p) j -> p c j", p=P))
            for c in range(4):
                gp.dma_start(
                    wv_ch[c][:, :, :],
                    wv_d[c * 2 * P:(c + 1) * 2 * P, :].rearrange(
                        "(c p) j -> p c j", p=P))

        def load_weights_rest():
            gp.dma_start(ws_t[:, :, :],
                         ws_d[:, :].rearrange("(c p) m -> p c m", p=P))
            if has_bias:
                nc.sync.dma_start(bg_sb[:, :], bg_d[:].rearrange("(c p) -> p c", p=P))
                nc.sync.dma_start(bv_sb[:, :], bv_d[:].rearrange("(c p) -> p c", p=P))
        if has_bias:
            bg_sb = singles.tile([P, JC], f32)
            bv_sb = singles.tile([P, JC], f32)
        magic = singles.tile([P, 1], i32)
        v.memset(magic[:, :], MAGIC)
        # preload the silu act table (Identity/Copy share it) at t~0
        warm = singles.tile([P, 1], f32)
        v.memset(warm[:, :], 0.0)
        if silu_mode == 'act':
            s.activation(out=warm[:, :], in_=warm[:, :], func=Act.Silu)
        # broadcastable constants (Pool engine can only tensor_tensor)
        cvals = {'half': 0.5, 'one': 1.0, 'i24': 1.0 / 24.0,
                 'i120': 1.0 / 120.0, 'i720': 1.0 / 720.0, 'i6': 1.0 / 6.0}
        consts = {}
        for cn, cv in cvals.items():
            ct = singles.tile([P, 1], f32, name=f"c_{cn}")
            v.memset(ct[:, :], cv)
            consts[cn] = ct
        # a16 staging buffers: pad rows memset ONCE, reused round-robin
        a16_bufs = []
        for i in range(2):
            a16 = singles.tile([MP, TG], f16, name=f"a16_{i}")
            gp.memset(a16[32:MP, :], 0.0)  # pool memsets: partition %32
            a16_bufs.append(a16)

        def rsqrt(y, ve, u, n, iters=2):
            """y <- rsqrt(ve), all [P, n] f32 tiles; u is scratch.

            Bit-trick seed + Newton; keeps sqrt off the scalar engine so it
            never swaps activation tables with Silu.
            """
            yi = y[:, 0:n].bitcast(i32)
            v.tensor_scalar(out=yi, in0=ve[:, 0:n].bitcast(i32), scalar1=1,
                            scalar2=None, op0=Alu.logical_shift_right)
            v.tensor_tensor(out=yi, in0=_fap(magic, 0, [[0, n]]), in1=yi,
                            op=Alu.subtract)
            for _ in range(iters):
                v.tensor_tensor(out=u[:, 0:n], in0=ve[:, 0:n], in1=y[:, 0:n],
                                op=Alu.mult)
                v.tensor_tensor(out=u[:, 0:n], in0=u[:, 0:n], in1=y[:, 0:n],
                                op=Alu.mult)
                v.tensor_scalar(out=u[:, 0:n], in0=u[:, 0:n], scalar1=-0.5,
                                scalar2=1.5, op0=Alu.mult, op1=Alu.add)
                v.tensor_tensor(out=y[:, 0:n], in0=y[:, 0:n], in1=u[:, 0:n],
                                op=Alu.mult)

        def tokr(t, s0, n, off=0, inner=None):
            """Per-token view of n expm slots starting at slot s0."""
            return _fap(t, s0 * MP + off, [[MP, n]] + (inner or [[1, M]]))

        def bmm_half(ops, eng, dst, a, b, s0, n, t1, t2, t3, final_add=None):
            """Queue dst = a @ b per 6x6 block for slots [s0, s0+n) on eng.

            Tree-structured adds: serial depth ~4 instead of 6."""
            def mul(o, k):
                aik = _fap(a, s0 * MP + k, [[MP, n], [6, 6], [0, 6]])
                bkj = _fap(b, s0 * MP + 6 * k, [[MP, n], [0, 6], [1, 6]])
                oij = _fap(o, s0 * MP, [[MP, n], [6, 6], [1, 6]])
                ops.append(lambda: eng.tensor_tensor(out=oij, in0=aik,
                                                     in1=bkj, op=Alu.mult))

            def add(o, i):
                ops.append(lambda: eng.tensor_tensor(
                    out=tokr(o, s0, n), in0=tokr(o, s0, n),
                    in1=tokr(i, s0, n), op=Alu.add))
            mul(dst, 0)
            mul(t1, 1)
            mul(t2, 2)
            mul(t3, 3)
            add(dst, t1)   # dst = p0+p1
            add(t2, t3)    # t2 = p2+p3
            mul(t1, 4)
            mul(t3, 5)
            add(t1, t3)    # t1 = p4+p5
            add(dst, t2)
            add(dst, t1)
            if final_add is not None:
                add(dst, final_add)

        def queue_expm(W, b, base, eg):
            """Build (ops_v, ops_gp) for clip + expm + store of batch b.

            The two lists are independent per-token-range chains."""
            sv = SV[eg]
            ops_pre, ops_v, ops_gp, ops_post = [], [], [], []
            GMX = EG_MAX * MP
            X = ep.tile([P, GMX], f16, tag="X")
            X2 = ep.tile([P, GMX], f16, tag="X2")
            X3 = ep.tile([P, GMX], f16, tag="X3")
            Q = ep.tile([P, GMX], f16, tag="Q")
            E = ep.tile([P, GMX], f16, tag="E")
            E2 = ep.tile([P, GMX], f16, tag="E2")
            tv = [ep.tile([P, GMX], f16, tag=f"tv{i}", name=f"tv{i}")
                  for i in range(3)]
            tg = [ep.tile([P, GMX], f16, tag=f"tg{i}", name=f"tg{i}")
                  for i in range(3)]
            Ef = efp.tile([P, GMX], f32, tag="Ef")
            fr = statp.tile([P, EG_MAX], f32, tag="fr")
            yb = statp.tile([P, EG_MAX], f32, tag="yb")
            ub = statp.tile([P, EG_MAX], f32, tag="ub")
            cs = statp.tile([P, EG_MAX], f16, tag="cs")

            def clip_a():
                # squared frobenius norm per token (v, all slots)
                v.tensor_tensor(out=tokr(tv[0], 0, eg), in0=tokr(W, 0, eg),
                                in1=tokr(W, 0, eg), op=Alu.mult)
                v.tensor_reduce(out=_fap(fr, 0, [[1, eg]]),
                                in_=tokr(tv[0], 0, eg),
                                axis=mybir.AxisListType.X, op=Alu.add)
                v.tensor_scalar_add(out=fr[:, 0:eg], in0=fr[:, 0:eg],
                                    scalar1=1e-30)
            ops_pre.append(clip_a)
            ops_pre.append(lambda: rsqrt(yb, fr, ub, eg))

            def clip_b():
                # c = min(frob, 3) * 2^-N_SQ / frob;  frob = fr * y
                v.tensor_tensor(out=ub[:, 0:eg], in0=fr[:, 0:eg],
                                in1=yb[:, 0:eg], op=Alu.mult)
                v.tensor_scalar(out=ub[:, 0:eg], in0=ub[:, 0:eg],
                                scalar1=MAX_NORM, scalar2=INV_SCALE,
                                op0=Alu.min, op1=Alu.mult)
                v.tensor_tensor(out=cs[:, 0:eg], in0=ub[:, 0:eg],
                                in1=yb[:, 0:eg], op=Alu.mult)
            ops_pre.append(clip_b)

            def xscale(eng, s0, n):
                return lambda: eng.tensor_tensor(
                    out=tokr(X, s0, n), in0=tokr(W, s0, n),
                    in1=_fap(cs, s0, [[1, n], [0, M]]), op=Alu.mult)
            ops_v.append(xscale(v, 0, sv))
            if sv < eg:
                ops_gp.append(xscale(gp, sv, eg - sv))

            def poly_v(ops, s0, n):
                # E = I + X + X2/2 ; Q = I/6 + X/24 + X2/120 + X3/720
                def f():
                    v.tensor_scalar(out=tokr(E, s0, n), in0=tokr(X2, s0, n),
                                    scalar1=0.5, scalar2=None, op0=Alu.mult)
                ops.append(f)
                ops.append(lambda: v.tensor_tensor(
                    out=tokr(E, s0, n), in0=tokr(E, s0, n),
                    in1=tokr(X, s0, n), op=Alu.add))
                ed = _fap(E, s0 * MP, [[MP, n], [7, 6]])
                ops.append(lambda: v.tensor_scalar_add(
                    out=ed, in0=ed, scalar1=1.0))
                ops.append(lambda: v.tensor_scalar(
                    out=tokr(Q, s0, n), in0=tokr(X, s0, n),
                    scalar1=1.0 / 24.0, scalar2=None, op0=Alu.mult))
                ops.append(lambda: v.scalar_tensor_tensor(
                    out=tokr(Q, s0, n), in0=tokr(X2, s0, n),
                    scalar=1.0 / 120.0, in1=tokr(Q, s0, n),
                    op0=Alu.mult, op1=Alu.add))
                ops.append(lambda: v.scalar_tensor_tensor(
                    out=tokr(Q, s0, n), in0=tokr(X3, s0, n),
                    scalar=1.0 / 720.0, in1=tokr(Q, s0, n),
                    op0=Alu.mult, op1=Alu.add))
                qd = _fap(Q, s0 * MP, [[MP, n], [7, 6]])
                ops.append(lambda: v.tensor_scalar_add(
                    out=qd, in0=qd, scalar1=1.0 / 6.0))

            def poly_gp(ops, s0, n, t):
                # same poly, but Pool only supports tensor_tensor: use
                # broadcast constant tiles and an explicit tmp
                def bc(cn):
                    return _fap(consts[cn], 0, [[0, n], [0, M]])

                def bcd(cn):
                    return _fap(consts[cn], 0, [[0, n], [0, 6]])

                def tt(o, a, bb):
                    return lambda: gp.tensor_tensor(out=o, in0=a, in1=bb,
                                                    op=Alu.mult)

                def ta(o, a, bb):
                    return lambda: gp.tensor_tensor(out=o, in0=a, in1=bb,
                                                    op=Alu.add)
                ops.append(tt(tokr(E, s0, n), tokr(X2, s0, n), bc('half')))
                ops.append(ta(tokr(E, s0, n), tokr(E, s0, n), tokr(X, s0, n)))
                ed = _fap(E, s0 * MP, [[MP, n], [7, 6]])
                ops.append(ta(ed, ed, bcd('one')))
                ops.append(tt(tokr(Q, s0, n), tokr(X, s0, n), bc('i24')))
                ops.append(tt(tokr(t, s0, n), tokr(X2, s0, n), bc('i120')))
                ops.append(ta(tokr(Q, s0, n), tokr(Q, s0, n), tokr(t, s0, n)))
                ops.append(tt(tokr(t, s0, n), tokr(X3, s0, n), bc('i720')))
                ops.append(ta(tokr(Q, s0, n), tokr(Q, s0, n), tokr(t, s0, n)))
                qd = _fap(Q, s0 * MP, [[MP, n], [7, 6]])
                ops.append(ta(qd, qd, bcd('i6')))

            def chain(ops, eng, s0, n, t1, t2, t3):
                bmm_half(ops, eng, X2, X, X, s0, n, t1, t2, t3)
                bmm_half(ops, eng, X3, X2, X, s0, n, t1, t2, t3)
                if eng is v:
                    poly_v(ops, s0, n)
                else:
                    poly_gp(ops, s0, n, t1)
                bmm_half(ops, eng, E2, X3, Q, s0, n, t1, t2, t3, final_add=E)
                assert N_SQ == 2
                bmm_half(ops, eng, E, E2, E2, s0, n, t1, t2, t3)
                bmm_half(ops, eng, Ef, E, E, s0, n, t1, t2, t3)
            chain(ops_v, v, 0, sv, *tv)
            if sv < eg:
                chain(ops_gp, gp, sv, eg - sv, *tg)

            base_tok = base * P
            out_v = out_d[base_tok:base_tok + sv * P, :].rearrange(
                "(g p) m -> p g m", p=P)
            # output DMAs on sync: gp's queue is a serial dependency
            # chain, and scalar must stay clear for silus.  Emitted only
            # after BOTH chains so the dep tracker sees the Ef writes.
            if sv < eg:
                out_g = out_d[base_tok + sv * P:base_tok + eg * P, :].rearrange(
                    "(g p) m -> p g m", p=P)
                ops_post.append(
                    lambda: nc.sync.dma_start(out_g, tokr(Ef, sv, eg - sv)))
            ops_post.append(lambda: nc.sync.dma_start(out_v, tokr(Ef, 0, sv)))
            return ops_pre, ops_v, ops_gp, ops_post

        # ---- pending expm work, paced into the next batch's groups ----
        # Emission ORDER is what the Tile dep tracker sees: 'pre' (clip,
        # writes cs on v) must be fully emitted before either chain, and
        # 'post' (out DMAs reading both Ef halves) strictly after both.
        pend = {'pre': [], 'v': [], 'g': [], 'post': [], 'post_prev': [],
                'nv': 0, 'ng': 0}

        def set_pending(ops):
            # out-DMAs of the batch before last carry over: emitted early
            # in the NEW batch so their semaphore-rotation slot can never
            # gate the boundary-critical transposes
            pend['post_prev'] = pend['post']
            pend['pre'], pend['v'], pend['g'], pend['post'] = ops
            pend['nv'], pend['ng'] = len(pend['v']), len(pend['g'])

        def emit_pending(frac):
            import math
            while pend['pre']:
                pend['pre'].pop(0)()
            for lst, n0 in ((pend['v'], pend['nv']), (pend['g'], pend['ng'])):
                target = math.ceil(n0 * min(frac, 1.0))
                while n0 - len(lst) < target:
                    lst.pop(0)()
            while pend['post_prev']:
                pend['post_prev'].pop(0)()

        def drain_pending():
            # proportional interleave so both engines keep working
            while pend['post_prev']:
                pend['post_prev'].pop(0)()
            while pend['pre']:
                pend['pre'].pop(0)()
            v_, g_ = pend['v'], pend['g']
            n0v, n0g = max(pend['nv'], 1), max(pend['ng'], 1)
            while v_ or g_:
                done_v = 1.0 - len(v_) / n0v
                done_g = 1.0 - len(g_) / n0g
                if v_ and (not g_ or done_v <= done_g):
                    v_.pop(0)()
                else:
                    g_.pop(0)()
            while pend['post']:
                pend['post'].pop(0)()

        xtas = {}        # group idx -> xta tile
        feed_state = {}  # group idx -> (xts, rst, negmur)
        W_tiles = {}     # batch idx -> W tile

        def emit_feeder_dma(G):
            """x DMAs for group G, issued 2 groups ahead so they are
            never queued behind transposes on sync."""
            xts = []
            for sub in range(GROUP):
                ti = G * GROUP + sub
                x_t = xp.tile([P, H], f16, tag=f"x{sub}")
                xts.append(x_t)
                nc.sync.dma_start(x_t[:, :], x_d[ti * P:(ti + 1) * P, :])
            feed_state[G] = xts

        def emit_feeder_stats(G):
            """LN stats + rsqrt for group G (DVE); x already landed."""
            xts = feed_state[G]
            mvg = statp.tile([P, GROUP, 2], f32, tag="mvg")
            for sub in range(GROUP):
                stats = statp.tile([P, 2, 6], f32, tag=f"st{sub}")
                xr = xts[sub][:, :].rearrange("p (a q) -> p a q", a=2)
                v.bn_stats(out=stats[:, 0, :], in_=xr[:, 0, :])
                v.bn_stats(out=stats[:, 1, :], in_=xr[:, 1, :])
                v.bn_aggr(out=mvg[:, sub, :], in_=stats[:, :, :])
            rst = statp.tile([P, GROUP], f32, tag="rst")
            ve = statp.tile([P, GROUP], f32, tag="ve")
            uu = statp.tile([P, GROUP], f32, tag="uu")
            negmur = statp.tile([P, GROUP], f32, tag="negmur")
            v.tensor_scalar_add(out=ve[:, :],
                                in0=_fap(mvg, 1, [[2, GROUP]]),
                                scalar1=LN_EPS)
            rsqrt(rst, ve, uu, GROUP)
            v.scalar_tensor_tensor(out=negmur[:, :],
                                   in0=_fap(mvg, 0, [[2, GROUP]]),
                                   scalar=-1.0, in1=rst[:, :],
                                   op0=Alu.mult, op1=Alu.mult)
            feed_state[G] = (xts, rst, negmur, mvg)

        def emit_feeder_norm(G):
            """normalize on scalar engine + XBAR transpose on sync."""
            xts, rst, negmur, mvg = feed_state.pop(G)
            xta = xtap.tile([P, KC, TG], f16, tag="xta")
            xtas[G] = xta
            tq = nc.sync if KOPT_TP_SYNC else nc.scalar
            for sub in range(GROUP):
                xh = xhp.tile([P, H], f16, tag=f"xh{sub}")
                if KOPT_NORM_ACT:
                    s.activation(out=xh[:, :], in_=xts[sub][:, :],
                                 func=Act.Identity,
                                 bias=negmur[:, sub:sub + 1],
                                 scale=rst[:, sub:sub + 1])
                else:
                    v.tensor_scalar(out=xh[:, :], in0=xts[sub][:, :],
                                    scalar1=mvg[:, sub, 0:1],
                                    scalar2=rst[:, sub:sub + 1],
                                    op0=Alu.subtract, op1=Alu.mult)
                tq.dma_start_transpose(
                    xta[:, :, sub * P:(sub + 1) * P], xh[:, :])

        def emit_compute(G, W, g, pace):
            """Matmuls + head for group G; W-slot g; pace(j0) interleaves
            pending expm ops and next group's feeder-norm."""
            xta = xtas.pop(G)
            h = [hp.tile([P, TG], f16, tag=f"h{j0}", name=f"h{j0}")
                 for j0 in range(JC)]
            pa = ps_pa.tile([M, TG], f32, tag="pa")
            for j0 in range(JC):
                pg = ps_mm.tile([P, TG], f32, tag="mm")
                for k0 in range(KC):
                    nc.tensor.matmul(
                        pg[:, :],
                        lhsT=wg_ch[k0 // 2][:, k0 % 2, j0 * P:(j0 + 1) * P],
                        rhs=xta[:, k0, :],
                        start=(k0 == 0), stop=(k0 == KC - 1))
                pv = ps_mm.tile([P, TG], f32, tag="mm")
                for k0 in range(KC):
                    nc.tensor.matmul(
                        pv[:, :],
                        lhsT=wv_ch[k0 // 2][:, k0 % 2, j0 * P:(j0 + 1) * P],
                        rhs=xta[:, k0, :],
                        start=(k0 == 0), stop=(k0 == KC - 1))
                if KOPT_HEAD_IL and j0 >= 1:
                    # head matmul for the previous chunk: h[j0-1] is ready
                    # by now, so the dependency never stalls the PE
                    nc.tensor.matmul(
                        pa[:, :], lhsT=ws_t[:, j0 - 1, :],
                        rhs=h[j0 - 1][:, :],
                        start=(j0 - 1 == 0), stop=False)
                sg = sgp.tile([P, TG], f32, tag="sg")
                gbias = bg_sb[:, j0:j0 + 1] if has_bias else 0.0
                if silu_mode == 'act':
                    s.activation(out=sg[:, :], in_=pg[:, :], func=Act.Silu,
                                 bias=gbias)
                else:
                    # silu(x) = x * sigmoid(x), for CoreSim (no Silu there)
                    s.activation(out=sg[:, :], in_=pg[:, :],
                                 func=Act.Sigmoid, bias=gbias)
                    if has_bias:
                        v.scalar_tensor_tensor(out=sg[:, :], in0=pg[:, :],
                                               scalar=bg_sb[:, j0:j0 + 1],
                                               in1=sg[:, :],
                                               op0=Alu.add, op1=Alu.mult)
                    else:
                        v.tensor_tensor(out=sg[:, :], in0=sg[:, :],
                                        in1=pg[:, :], op=Alu.mult)
                if has_bias:
                    v.tensor_scalar(out=h[j0][:, :], in0=pv[:, :],
                                    scalar1=bv_sb[:, j0:j0 + 1],
                                    scalar2=None, op0=Alu.add)
                    v.tensor_tensor(out=h[j0][:, :], in0=h[j0][:, :],
                                    in1=sg[:, :], op=Alu.mult)
                else:
                    v.tensor_tensor(out=h[j0][:, :], in0=sg[:, :],
                                    in1=pv[:, :], op=Alu.mult)
                pace(j0)
            if KOPT_HEAD_IL:
                nc.tensor.matmul(pa[:, :], lhsT=ws_t[:, JC - 1, :],
                                 rhs=h[JC - 1][:, :], start=False, stop=True)
            else:
                for j0 in range(JC):
                    nc.tensor.matmul(pa[:, :], lhsT=ws_t[:, j0, :],
                                     rhs=h[j0][:, :],
                                     start=(j0 == 0), stop=(j0 == JC - 1))
            a16 = a16_bufs[G % 2]
            if KOPT_A16_ACT:
                s.activation(out=a16[0:M, :], in_=pa[:, :], func=Act.Copy)
            else:
                v.tensor_copy(out=a16[0:M, :], in_=pa[:, :])
            # token-major [128, GROUP, MP] slice of the batch W tile
            wv_ = _fap(W, g * GROUP * MP, [[MP, GROUP], [1, MP]])
            (nc.sync if KOPT_TP_SYNC else nc.scalar).dma_start_transpose(
                wv_, a16[:, :])

        # ---- main schedule ----
        emit_feeder_dma(0)
        load_weights_main()
        emit_feeder_dma(1)
        load_weights_rest()
        emit_feeder_stats(0)
        emit_feeder_norm(0)
        G = 0
        base = 0
        for b, eg in enumerate(egs):
            NG = eg // GROUP
            W_tiles[b] = wp.tile([P, EG_MAX * MP], f16, tag="W", name="W")
            for g in range(NG):
                if G + 2 < NGT:
                    emit_feeder_dma(G + 2)
                if G + 1 < NGT:
                    # stats, then norm+transpose, all at the group FRONT:
                    # the Act queue must never have silus queued behind
                    # v-dependent norms mid-group (lag feedback loop)
                    emit_feeder_stats(G + 1)
                    emit_feeder_norm(G + 1)
                slots = NG * 6

                def pace(j0, g=g, NG=NG, slots=slots, G=G):
                    if 1 <= j0 <= 6:
                        # mild front-load: emission done ~5/6 through the
                        # batch; chains have an extra batch of slack (ep
                        # pool is double-buffered)
                        emit_pending(1.2 * (g * 6 + j0) / slots)
                emit_compute(G, W_tiles[b], g, pace)
                G += 1
            drain_pending()  # safety: should already be empty
            set_pending(queue_expm(W_tiles.pop(b), b, base, eg))
            base += eg
        drain_pending()

    return nc


def split_waits(nc, ctrl_limit=1, limit=1):
    """Walrus codegen caps sem-wait commands per instruction (1 for CTRL-type
    Drain/NoOp). Hoist excess waits onto preceding same-engine NoOps."""
    import concourse.mybir as mybir
    n = 0
    for fn in nc.m.functions:
        for blk in fn.blocks:
            out = []
            changed = False
            for inst in blk.instructions:
                lim = (ctrl_limit if inst.opcode in
                       ("Drain", "NoOp", "EventSemaphoreOp") else limit)
                si = inst.sync_info
                waits = list(si.on_wait) if si is not None and si.on_wait else []
                if len(waits) > lim:
                    head, tail = waits[:-lim], waits[-lim:]
                    k = 0
                    while head:
                        chunk, head = head[:ctrl_limit], head[ctrl_limit:]
                        out.append(mybir.InstNoOp(
                            name=f"{inst.name}-wsplit{k}",
                            engine=inst.engine, ins=[], outs=[],
                            sync_info=mybir.SyncInfo(on_wait=chunk, on_update=[]),
                        ))
                        k += 1
                        n += 1
                    si.on_wait = tail
                    changed = True
                out.append(inst)
            if changed:
                blk.instructions = out
    return n


def _build_program(b_shard, has_bias):
    import concourse.bass as bass
    nc = bass.Bass()
    build(nc, b_shard, has_bias=has_bias)
    split_waits(nc)
    return nc


def prepare_inputs(state_token, gamma, beta, w_gate, w_val, w_out):
    """Host-side preprocessing: fold gamma/beta into weights, fold the
    skew-symmetrization into w_out, cast to fp16."""
    gamma = np.asarray(gamma, np.float32)
    beta = np.asarray(beta, np.float32)
    w_gate = np.asarray(w_gate, np.float32)
    w_val = np.asarray(w_val, np.float32)
    w_out = np.asarray(w_out, np.float32)
    wgT = np.ascontiguousarray((w_gate * gamma).T).astype(np.float16)
    wvT = np.ascontiguousarray((w_val * gamma).T).astype(np.float16)
    # skew fold: As[t,(i,j)] = sum_k h[t,k] * (w_out[(i,j),k] - w_out[(j,i),k])
    w_o66 = w_out.reshape(6, 6, H)
    ws = (w_o66 - np.swapaxes(w_o66, 0, 1)).reshape(M, H)
    wsT = np.ascontiguousarray(ws.T).astype(np.float16)
    bg = w_gate @ beta
    bv = w_val @ beta
    has_bias = bool(np.any(bg) or np.any(bv))
    return wgT, wvT, wsT, bg, bv, has_bias


def kernel(state_token, gamma, beta, w_gate, w_val, w_out):
    from concourse.bass_utils import run_bass_kernel_spmd

    x = np.asarray(state_token, np.float32)
    assert x.shape == (B_FULL, H), x.shape
    x16 = x.astype(np.float16)
    wgT, wvT, wsT, bg, bv, has_bias = prepare_inputs(
        state_token, gamma, beta, w_gate, w_val, w_out)

    nc = _build_program(B_SHARD, has_bias)

    in_maps = []
    for i in range(N_CORES):
        m = {
            "x": np.ascontiguousarray(x16[i * B_SHARD:(i + 1) * B_SHARD]),
            "wgT": wgT, "wvT": wvT, "wsT": wsT,
        }
        if has_bias:
            m["bg"] = bg
            m["bv"] = bv
        in_maps.append(m)

    import os
    tmpdir = os.environ.get("KERNEL_TRACE_DIR") or None
    res = run_bass_kernel_spmd(nc, in_maps, list(range(N_CORES)), tmpdir=tmpdir)
    kernel._last = res  # for test harness introspection (exec time, trace)
    out = np.concatenate([res.results[i]["out"] for i in range(N_CORES)], axis=0)
    return out.reshape(B_FULL, 6, 6)
